# revision 4
# baseline (speedup 1.0000x reference)
"""Trainium2 Bass kernel for the three-GEU (text/video/audio) embedding model.

Strategy (8 NeuronCores, one chip):
  - Tensor-parallel column sharding: core c owns output columns [512c, 512(c+1))
    of every linear; it reads only its 1/8 slice of each weight matrix.
  - Preprocessing (text max-pool over L, audio ragged masked-mean over T) is
    sharded over the feature dim, computed in transposed layout, and an
    AllGather assembles the full [K, B] activations every core needs as the
    matmul stationary operand.
  - Each GEU: GEMM1 -> PE-transpose h -> AllGather(hT) -> GEMM2 -> sigmoid,
    y = h * sig(g), partial sum(y^2); AllGather of the three norm partials
    (+ local sum) then rsqrt scaling on-device.
  - DMA discipline: all activation DMAs are issued on the two HWDGE rings
    BEFORE any weight chunk, so the pooled activations (and hence the first
    AllGather) are ready ~10us in instead of starving behind 22 MiB of
    weight traffic.
"""

import numpy as np

B = 64
L = 30
D = 4096
DA = 1024
T = 128
NCORES = 8
S = D // NCORES     # 512: per-core output shard of D
SA = DA // NCORES   # 128: per-core shard of Da
KD = D // 128       # 32 k-tiles over D
KA = DA // 128      # 8 k-tiles over Da
CH = 16             # k-tiles per weight DMA chunk

_STATE: dict = {}


def _build():
    from contextlib import ExitStack

    import concourse.bass as bass
    import concourse.tile as tile
    from concourse import bacc, mybir
    from concourse.bass import ts
    from concourse.masks import make_identity

    fp16 = mybir.dt.float16
    f32 = mybir.dt.float32
    AX = mybir.AxisListType
    ALU = mybir.AluOpType
    ACTF = mybir.ActivationFunctionType

    nc = bacc.Bacc(
        "TRN2",
        target_bir_lowering=False,
        debug=False,
        enable_asserts=False,
        num_devices=NCORES,
    )
    RG = [list(range(NCORES))]

    # --- kernel I/O (per-core shards, staged by the host wrapper) ---
    # Weights arrive pre-tiled [n_chunks, 128, CH*S] so each chunk DMA is
    # fully contiguous per partition.
    w_in = {}
    for name, kk in [("wt", D), ("wgt", D), ("wv", D), ("wgv", D),
                     ("wga", D), ("wa", DA)]:
        nkt = kk // 128
        nch = max(1, nkt // CH)
        w_in[name] = nc.dram_tensor(
            name, [nch, 128, (nkt // nch) * S], fp16, kind="ExternalInput")
    textT = nc.dram_tensor("textT", [S, B, L], fp16, kind="ExternalInput")
    audioT = nc.dram_tensor("audioT", [T, B, SA], fp16, kind="ExternalInput")
    vT_d = nc.dram_tensor("vT", [128, KD, B], fp16, kind="ExternalInput")
    maskT_d = nc.dram_tensor("maskT", [T, B], fp16, kind="ExternalInput")
    biases_d = nc.dram_tensor("biases", [1, 6 * S], fp16, kind="ExternalInput")
    EMBEDS = ("text", "video", "audio")
    out_d = {
        e: nc.dram_tensor(f"out_{e}", [B, S], f32, kind="ExternalOutput")
        for e in EMBEDS
    }

    BIAS_IDX = {("text", 1): 0, ("text", 2): 1, ("video", 1): 2,
                ("video", 2): 3, ("audio", 1): 4, ("audio", 2): 5}

    with ExitStack() as ctx:
        tc = ctx.enter_context(tile.TileContext(nc))

        persist = ctx.enter_context(tc.tile_pool(name="persist", bufs=1))
        wpool = ctx.enter_context(tc.tile_pool(name="wstream", bufs=6))
        work = ctx.enter_context(tc.tile_pool(name="work", bufs=2))
        psum = ctx.enter_context(tc.tile_pool(name="psum", bufs=2, space="PSUM"))
        dram = ctx.enter_context(tc.tile_pool(name="dram", bufs=1, space="DRAM"))

        # ---- persistent SBUF tiles ----
        acts_all = persist.tile([128, 40, B], fp16)   # AG1: 8 chunks x (4 text + 1 audio)
        au_sb = persist.tile([T, B, SA], fp16)        # audio shard, [t, b, c']
        vt_sb = persist.tile([128, KD, B], fp16)      # video.T k-tiles
        msk_sb = persist.tile([T, B], fp16)           # mask/nf, transposed
        bias_sb = persist.tile([1, 6, S], fp16)
        ones_sb = persist.tile([1, B], fp16)
        ident = persist.tile([B, B], fp16)
        stg = persist.tile([128, 5, B], fp16)         # AG1 staging
        nsq = persist.tile([B, 4], f32)               # partial sum(y^2) per embed
        nsqg = persist.tile([B, NCORES, 3], f32)      # gathered partials
        nsum = persist.tile([B, 3], f32)
        nrm = persist.tile([B, 3], f32)
        rcp = persist.tile([B, 3], f32)
        # combined transposed-h staging / gathered result (one AllGather)
        hstg = persist.tile([128, 3, 4, B], fp16)
        hT_all = persist.tile([128, NCORES, 3, 4, B], fp16)
        h16 = {e: persist.tile([B, S], fp16, name=f"h16_{e}") for e in EMBEDS}
        y_sb = {e: persist.tile([B, S], f32, name=f"y_{e}") for e in EMBEDS}

        # ---- constants (gpsimd+vector; no DMA rings involved) ----
        nc.gpsimd.memset(ones_sb[:], 1.0)
        nc.vector.memset(nsq[:], 0.0)
        make_identity(nc, ident[:])

        # ---- activation DMAs FIRST on both HWDGE rings, weights after ----
        # sync ring: audio, mask, text tiles 0-1
        # scalar ring: bias, video, text tiles 2-3
        nc.sync.dma_start(au_sb[:], audioT.ap())
        nc.sync.dma_start(msk_sb[:], maskT_d.ap())
        nc.scalar.dma_start(bias_sb[0:1, :, :], biases_d.ap())
        nc.scalar.dma_start(vt_sb[:], vT_d.ap())
        t_view = textT.ap().rearrange("(n p) b l -> n p b l", p=128)
        tx = [work.tile([128, B, L], fp16, name=f"tx{i}", bufs=1)
              for i in range(4)]
        for i in range(4):
            eng = nc.sync if i < 2 else nc.scalar
            eng.dma_start(tx[i][:], t_view[i])

        # ---- text max-pool over L (sharded over d) -> stg[:, 0:4, :] ----
        for i in range(4):
            nc.vector.reduce_max(stg[:, i, :], tx[i][:], AX.X)

        # ---- audio ragged masked-mean (sharded over Da): 64 PE matvecs ----
        aT_ps = psum.tile([SA, B], f32, bufs=1)
        for b in range(B):
            nc.tensor.matmul(
                aT_ps[:, b:b + 1], au_sb[:, b, :], msk_sb[:, b:b + 1],
                start=True, stop=True)
        nc.vector.tensor_copy(stg[:, 4, :], aT_ps[:])

        # ---- AllGather the preprocessed activations ----
        ag1_in = dram.tile([128, 5 * B], fp16)
        ag1_out = dram.tile([128 * NCORES, 5 * B], fp16, addr_space="Shared")
        nc.gpsimd.dma_start(ag1_in[:], stg[:])
        nc.gpsimd.collective_compute(
            "AllGather", ALU.bypass, replica_groups=RG,
            ins=[ag1_in.opt()], outs=[ag1_out.opt()])
        nc.gpsimd.dma_start(
            acts_all.rearrange("p (r s) b -> p r (s b)", s=5),
            ag1_out.rearrange("(r p) sb -> p r sb", p=128))

        # lhsT accessors (stationary [128, B] k-tiles, transposed activations)
        def lhs_text(k):
            return acts_all[:, (k // 4) * 5 + (k % 4), :]

        def lhs_audio(k):
            return acts_all[:, k * 5 + 4, :]

        def lhs_video(k):
            return vt_sb[:, k, :]

        hwdge = [nc.sync, nc.scalar]
        chunk_no = [0]

        def gemm(out_ps, w_dram, n_kt, lhs_fn, bias_idx):
            # bias as a K=1 matmul row; also opens the accumulation group
            nc.tensor.matmul(out_ps[:], ones_sb[:], bias_sb[:, bias_idx, :],
                             start=True, stop=False)
            nch = w_dram.shape[0]
            cnt = n_kt // nch
            for ch in range(nch):
                w = wpool.tile([128, cnt, S], fp16, name="wchunk", tag="wchunk")
                eng = hwdge[chunk_no[0] % 2]
                chunk_no[0] += 1
                eng.dma_start(
                    w[:], w_dram.ap()[ch].rearrange("p (a n) -> p a n", n=S))
                for a in range(cnt):
                    k = ch * cnt + a
                    nc.tensor.matmul(out_ps[:], lhs_fn(k), w[:, a, :],
                                     start=False, stop=(k == n_kt - 1))

        # ---- stage 1: three first linears (video first: no AG1 dep) ----
        W1 = {"video": ("wv", KD, lhs_video), "text": ("wt", KD, lhs_text),
              "audio": ("wa", KA, lhs_audio)}
        EORD = ("video", "text", "audio")
        EIDX = {e: i for i, e in enumerate(EMBEDS)}
        for e in EORD:
            wname, nkt, lf = W1[e]
            ei = EIDX[e]
            h_ps = psum.tile([B, S], f32, name="h_ps", tag="h_ps")
            gemm(h_ps, w_in[wname], nkt, lf, BIAS_IDX[(e, 1)])
            nc.vector.tensor_copy(h16[e][:], h_ps[:])
            # transpose h shard ([B, S] -> 4 x [128, B]) on the PE
            hT_ps = psum.tile([128, 4, B], fp16, name="hT_ps", tag="hT_ps",
                              bufs=1)
            for j in range(4):
                nc.tensor.transpose(hT_ps[:, j, :], h16[e][:, ts(j, 128)],
                                    ident[:])
            nc.vector.tensor_copy(hstg[:, ei, :, :], hT_ps[:])

        # ---- AllGather hT for all three GEUs at once ----
        agh_in = dram.tile([128, 3 * 4 * B], fp16)
        agh_out = dram.tile([128 * NCORES, 3 * 4 * B], fp16,
                            addr_space="Shared")
        nc.gpsimd.dma_start(agh_in[:], hstg[:])
        nc.gpsimd.collective_compute(
            "AllGather", ALU.bypass, replica_groups=RG,
            ins=[agh_in.opt()], outs=[agh_out.opt()])
        nc.gpsimd.dma_start(
            hT_all.rearrange("p r e j b -> p r (e j b)"),
            agh_out.rearrange("(r p) x -> p r x", p=128))

        # ---- stage 2: gating linears, GLU, partial norms ----
        W2 = {"text": "wgt", "video": "wgv", "audio": "wga"}
        for e in EORD:
            ei = EIDX[e]
            g_ps = psum.tile([B, S], f32, name="g_ps", tag="g_ps")
            gemm(g_ps, w_in[W2[e]], KD,
                 lambda k, ei=ei: hT_all[:, k // 4, ei, k % 4, :],
                 BIAS_IDX[(e, 2)])
            sg16 = work.tile([B, S], fp16, name="sg16", tag="sg16")
            nc.scalar.activation(sg16[:], g_ps[:], ACTF.Sigmoid)
            nc.vector.tensor_mul(y_sb[e][:], h16[e][:], sg16[:])
            ysq = work.tile([B, S], f32, name="ysq", tag="ysq")
            nc.vector.tensor_mul(ysq[:], y_sb[e][:], y_sb[e][:])
            nc.vector.reduce_sum(nsq[:, ei:ei + 1], ysq[:], AX.X)

        # ---- AllGather norm partials; sum locally; normalize; write out ----
        ar_in = dram.tile([B, 3], f32)
        ar_out = dram.tile([B * NCORES, 3], f32, addr_space="Shared")
        nc.gpsimd.dma_start(ar_in[:], nsq[:, 0:3])
        nc.gpsimd.collective_compute(
            "AllGather", ALU.bypass, replica_groups=RG,
            ins=[ar_in.opt()], outs=[ar_out.opt()])
        nc.gpsimd.dma_start(
            nsqg[:], ar_out.rearrange("(r p) x -> p r x", p=B))
        nc.vector.tensor_add(nsum[:], nsqg[:, 0, :], nsqg[:, 1, :])
        for r in range(2, NCORES):
            nc.vector.tensor_add(nsum[:], nsum[:], nsqg[:, r, :])
        nc.scalar.sqrt(nrm[:], nsum[:])
        nc.vector.tensor_scalar_max(nrm[:], nrm[:], 1e-12)
        nc.vector.reciprocal(rcp[:], nrm[:])
        for e in EORD:
            ei = EIDX[e]
            yo = work.tile([B, S], f32, name="yo", tag="yo")
            nc.vector.tensor_scalar_mul(yo[:], y_sb[e][:],
                                        rcp[:, ei:ei + 1])
            nc.sync.dma_start(out_d[e].ap(), yo[:])

    nc.compile()
    return nc


def _get_nc():
    if "nc" not in _STATE:
        _STATE["nc"] = _build()
    return _STATE["nc"]


def _prep_inputs(text, video, audio_feats, Wt, bt, Wgt, bgt, Wv, bv, Wgv, bgv,
                 Wa, ba, Wga, bga, nframes, raw_audio_len):
    """Shard + transpose + fp16-cast the full inputs into per-core in_maps."""
    f16 = np.float16
    text = np.asarray(text, dtype=np.float32)
    video = np.asarray(video, dtype=np.float32)
    audio = np.asarray(audio_feats, dtype=np.float32)

    ratio = int(round(float(np.asarray(raw_audio_len)) / T))
    nf = np.maximum(
        1, (np.asarray(nframes).astype(np.float32) / ratio).astype(np.int32))
    mask = (np.arange(T)[None, :] < nf[:, None]).astype(np.float32)
    mask = mask / nf[:, None].astype(np.float32)          # [B, T] mask/nf
    maskT = np.ascontiguousarray(mask.T).astype(f16)      # [T, B]

    # video.T pre-tiled to [128, KD, B] (partition-contiguous k-tiles)
    vT = np.ascontiguousarray(
        video.T.reshape(KD, 128, B).transpose(1, 0, 2)).astype(f16)

    def wtile(W, sl):
        """W[sl].T [K, S] -> chunked [nch, 128, cnt*S], contiguous/partition."""
        wt = W[sl, :].T
        kk = wt.shape[0]
        nkt = kk // 128
        nch = max(1, nkt // CH)
        cnt = nkt // nch
        return np.ascontiguousarray(
            wt.reshape(nch, cnt, 128, S).transpose(0, 2, 1, 3)
            .reshape(nch, 128, cnt * S)).astype(f16)

    in_maps = []
    for c in range(NCORES):
        sl = slice(c * S, (c + 1) * S)
        sla = slice(c * SA, (c + 1) * SA)
        m = {
            "wt": wtile(Wt, sl),
            "wgt": wtile(Wgt, sl),
            "wv": wtile(Wv, sl),
            "wgv": wtile(Wgv, sl),
            "wga": wtile(Wga, sl),
            "wa": wtile(Wa, sl),
            "textT": np.ascontiguousarray(
                text[:, :, sl].transpose(2, 0, 1)).astype(f16),
            "audioT": np.ascontiguousarray(
                audio[:, sla, :].transpose(2, 0, 1)).astype(f16),
            "vT": vT,
            "maskT": maskT,
            "biases": np.stack([
                np.asarray(b, dtype=np.float32)[sl] for b in
                (bt, bgt, bv, bgv, ba, bga)
            ]).reshape(1, -1).astype(f16),
        }
        in_maps.append(m)
    return in_maps


def kernel(text, video, audio_feats, Wt, bt, Wgt, bgt, Wv, bv, Wgv, bgv,
           Wa, ba, Wga, bga, nframes, raw_audio_len):
    from concourse.bass_utils import run_bass_kernel_spmd

    nc = _get_nc()
    in_maps = _prep_inputs(text, video, audio_feats, Wt, bt, Wgt, bgt,
                           Wv, bv, Wgv, bgv, Wa, ba, Wga, bga,
                           nframes, raw_audio_len)
    res = run_bass_kernel_spmd(nc, in_maps, list(range(NCORES)))
    _STATE["last_results"] = res
    outs = []
    for e in ("text", "video", "audio"):
        outs.append(np.concatenate(
            [res.results[c][f"out_{e}"] for c in range(NCORES)], axis=1))
    return tuple(outs)
